# revision 21
# baseline (speedup 1.0000x reference)
"""Trainium2 Bass kernel: batched HMM log-forward (evidence) via segmented
rank-1 scan.

Problem: B=128 seqs, T=8192 steps, S=65 states (state 0 bookend), V=1024.
Linear-space chain:  Z*e^99 = w~^T A_8191 ... A_1 a_1,
  A_t = D_{e_t} Ttil^T,  a_1 = e_0*trans0,  w~ = exp(log_trans[1:,0]+99),
  e_t = E[:, obs[t]],    E = exp(log_emit[1:,:] + C)  (drift-compensated).

Key idea: a product of >=130 positive matrices is numerically rank-1
(Perron-Frobenius contraction; validated offline to ~1e-6), so the chain
splits into NPAIRS+1 blocks stitched by 64-dim dot products:
  * block 0 (exact fwd from a_1), block NPAIRS (exact bwd from w~), and
    NPAIRS-1 interior blocks, each contributing one fwd chain (direction
    u_k) and one bwd chain (direction v_k) from arbitrary positive inits.
  * stitch right-to-left: acc^T M_k = (acc.f_k)/(h_k.g_k) * h_k^T, where
    h_k = Ttil @ (device bwd output)  [the device bwd chain computes the
    T~-shifted product; one host matvec restores M_k^T r_k].
Serial depth drops 4096 -> 130 steps; the extra chains ride in the matmul
columns (PE/DVE have huge column headroom at the latency floor).

Device step (per chain group): one [128x128]@[128,cw] matmul -> PSUM, one
DVE multiply PSUM*e -> SBUF.  Emission stream is fp8 e5m2 (range of E is
[6e-3, 37] - all normal in e5m2; validated 4.7e-4 end-to-end), so the
entire 130-step stream fits in SBUF upfront (128KB/partition).

Sharding: data-parallel, 16 seqs per core on 8 cores.  Host prep builds
per-core streams; host epilogue stitches in float64 (microseconds).
"""

import os
import numpy as np
import ml_dtypes

B, T, S, V = 128, 8192, 65, 1024
N_CORES = 8
SEQ_PER_CORE = B // N_CORES  # 16
C_SHIFT = 6.9418
BF16 = ml_dtypes.bfloat16
FP8 = ml_dtypes.float8_e5m2

# segmentation: matrices t in [1, 8191] split into N_PAIRS+1 blocks -- the
# exact-bwd block consumes N_STEPS+1 matrices (its init absorbs one
# emission), every other block N_STEPS.
N_STEPS = int(os.environ.get("HMM_NSTEPS", "65"))
N_PAIRS = 8190 // N_STEPS - 1  # 125 for N_STEPS=65
assert (N_PAIRS + 1) * N_STEPS == 8190
NCOL = N_PAIRS * SEQ_PER_CORE  # device columns per core (1008)


def _chain_ranges(n_chains):
    """Split the NCOL columns into chain groups on pair boundaries. A
    group's PSUM tile may span multiple 2KB banks (512 fp32 cols each);
    individual matmuls are split at bank boundaries by the builder."""
    base = N_PAIRS // n_chains
    rem = N_PAIRS % n_chains
    sizes = [(base + (1 if i < rem else 0)) * SEQ_PER_CORE for i in range(n_chains)]
    ranges = []
    lo = 0
    for s in sizes:
        ranges.append((lo, lo + s))
        lo += s
    return ranges


def _strip_self_wait_events(nc):
    """Remove InstEventSemaphore instrs that only wait on the issuing
    engine's own semaphore (trivially-true WAW guards; engine execution is
    in-order). Saves sequencer slots in the scan loop."""
    eng_prefix = {
        "EngineType.DVE": "DVE_",
        "EngineType.PE": "PE_",
        "EngineType.Activation": "Activation_",
        "EngineType.Pool": "Pool_",
    }
    removed = 0
    for fn in nc.m.functions:
        for blk in fn.blocks:
            keep = []
            for inst in blk.instructions:
                if type(inst).__name__ == "InstEventSemaphore":
                    pfx = eng_prefix.get(str(getattr(inst, "engine", "")), None)
                    si = inst.sync_info
                    if (
                        pfx is not None
                        and si
                        and not si.on_update
                        and si.on_wait
                        and all(
                            w.ant_name.startswith(pfx)
                            and w.wait_mode == "sem-ge-imm"
                            for w in si.on_wait
                        )
                    ):
                        removed += 1
                        continue
                keep.append(inst)
            blk.instructions[:] = keep
    return removed


def _dedupe_ldweights(nc):
    """Drop InstLdweights reloading the identical stationary operand the PE
    already holds (weight never changes across the scan)."""
    removed = 0
    for fn in nc.m.functions:
        for blk in fn.blocks:
            last_key = None
            keep = []
            for inst in blk.instructions:
                if type(inst).__name__ == "InstLdweights":
                    si = inst.sync_info
                    clean = not si or (not si.on_wait and not si.on_update)
                    key = (
                        str(inst.ins[0]),
                        str(getattr(inst, "tile_position", None)),
                        str(getattr(inst, "perf_mode", None)),
                    )
                    if clean and key == last_key:
                        removed += 1
                        continue
                    if clean:
                        last_key = key
                    else:
                        last_key = None
                keep.append(inst)
            blk.instructions[:] = keep
    return removed


def _build_program(n_steps, n_chains):
    """SPMD Bass program: n_steps scan iterations over NCOL columns."""
    import contextlib
    import concourse.tile as tile
    from concourse import bacc, mybir

    nc = bacc.Bacc(None)
    ranges = _chain_ranges(n_chains)
    # PSUM banks hold 512 fp32 columns; a chain tile may span several banks
    # (matmul outputs must not cross a bank boundary, so split them there)
    max_cw = max(hi - lo for lo, hi in ranges)
    banks_per_tile = -(-max_cw // 512)
    psum_bufs = 2 if n_chains * banks_per_tile * 2 <= 8 else 1

    w_dram = nc.declare_dram_parameter("wmat", [128, 128], mybir.dt.bfloat16, False)
    x0_dram = nc.declare_dram_parameter("x0", [128, NCOL], mybir.dt.bfloat16, False)
    e_dram = nc.declare_dram_parameter(
        "econg", [128, n_steps * NCOL], mybir.dt.float8e5, False
    )
    out_dram = nc.declare_dram_parameter("xout", [128, NCOL], mybir.dt.bfloat16, True)

    # e-stream chunk schedule (steps per chunk): small first chunks so the
    # scan starts early, then large ones
    sched = []
    left = n_steps
    for sz in (1, 2, 4, 8):
        if left > 0:
            sched.append(min(sz, left))
            left -= sched[-1]
    while left > 0:
        sched.append(min(16, left))
        left -= sched[-1]
    chunk_of_step = []
    for ci, sz in enumerate(sched):
        chunk_of_step += [ci] * sz
    chunk_base = np.cumsum([0] + sched[:-1])

    with tile.TileContext(nc) as tc:
        with contextlib.ExitStack() as ctx:
            const_pool = ctx.enter_context(tc.tile_pool(name="const", bufs=1))
            epool = ctx.enter_context(tc.tile_pool(name="emis", bufs=1))
            xpool = ctx.enter_context(tc.tile_pool(name="x", bufs=4))
            psum_pool = ctx.enter_context(
                tc.tile_pool(name="ps", bufs=psum_bufs, space="PSUM")
            )
            fin_pool = ctx.enter_context(tc.tile_pool(name="fin", bufs=1))

            w_sb = const_pool.tile([128, 128], mybir.dt.bfloat16, tag="w")
            nc.gpsimd.dma_start(w_sb[:], w_dram[:])
            # per-chain x0 tiles: chain A's first matmul starts as soon as
            # its own init columns land, not after the full x0 transfer
            x0_tiles = []
            for ch, (lo, hi) in enumerate(ranges):
                xt = const_pool.tile(
                    [128, hi - lo], mybir.dt.bfloat16, tag=f"x0{ch}"
                )
                nc.gpsimd.dma_start(xt[:], x0_dram[:, lo:hi])
                x0_tiles.append(xt)

            e_tiles = []
            for ci, sz in enumerate(sched):
                et = epool.tile([128, sz * NCOL], mybir.dt.float8e5, tag=f"e{ci}")
                lo = int(chunk_base[ci]) * NCOL
                nc.gpsimd.dma_start(et[:], e_dram[:, lo : lo + sz * NCOL])
                e_tiles.append(et)

            dummy = fin_pool.tile([1, 4], mybir.dt.bfloat16, tag="dummy")

            xs = [(x0_tiles[ch], 0) for ch in range(len(ranges))]
            seen_chunk = -1
            for k in range(n_steps):
                ci = chunk_of_step[k]
                off = (k - int(chunk_base[ci])) * NCOL
                if ci != seen_chunk:
                    nc.vector.tensor_copy(dummy[0:1, 0:1], e_tiles[ci][0:1, 0:1])
                    seen_chunk = ci
                for ch, (lo, hi) in enumerate(ranges):
                    cw = hi - lo
                    xt, xo = xs[ch]
                    ps = psum_pool.tile([128, cw], mybir.dt.float32, tag=f"ps{ch}")
                    for sub in range(0, cw, 512):
                        se = min(cw, sub + 512)
                        nc.tensor.matmul(
                            ps[:, sub:se],
                            w_sb[:],
                            xt[:, xo + sub : xo + se],
                            start=True,
                            stop=True,
                        )
                    xn = xpool.tile([128, cw], mybir.dt.bfloat16, tag=f"x{ch}")
                    nc.vector.tensor_mul(
                        xn[:], ps[:], e_tiles[ci][:, off + lo : off + hi]
                    )
                    xs[ch] = (xn, 0)

            for ch, (lo, hi) in enumerate(ranges):
                xt, xo = xs[ch]
                nc.sync.dma_start(out_dram[:, lo:hi], xt[:, xo : xo + (hi - lo)])

    nc.compile()
    _dedupe_ldweights(nc)
    if os.environ.get("HMM_STRIP_EV", "1") == "1":
        _strip_self_wait_events(nc)
    return nc


def _host_prep(log_trans, log_emit, obvs, n_steps):
    """Per-core device inputs: weight, init columns, fp8 emission stream."""
    log_trans = np.asarray(log_trans, dtype=np.float64)
    log_emit = np.asarray(log_emit, dtype=np.float64)
    obvs = np.asarray(obvs).astype(np.int64)

    Ttil = np.exp(log_trans[1:, 1:])
    trans0 = np.exp(log_trans[0, 1:])
    w_til = np.exp(log_trans[1:, 0] + 99.0)
    E = np.exp(log_emit[1:, :] + C_SHIFT)  # [64,1024]
    E8 = E.astype(FP8)

    wmat = np.zeros((128, 128), dtype=np.float64)
    wmat[0:64, 0:64] = Ttil
    wmat[64:128, 64:128] = Ttil.T
    wmat = wmat.astype(BF16)

    npair = N_PAIRS
    # token index tables, shape [npair, n_steps], step i=1..n_steps
    i_idx = np.arange(1, n_steps + 1)
    j_idx = np.arange(npair)
    fwd_tok = n_steps * j_idx[:, None] + i_idx[None, :]  # block j: t=n*j+i
    # bwd block l=j+1: interior (l<npair): t = s_{l+1}-i = n*(l+1)+1-i
    l_idx = j_idx + 1
    bwd_tok = n_steps * (l_idx[:, None] + 1) + 1 - i_idx[None, :]
    bwd_tok[-1, :] = (T - 1) - i_idx  # exact bwd block: t = T-1-i

    per_core = []
    uni = np.full(64, 1.0 / 64, dtype=np.float64)
    for m in range(N_CORES):
        s0 = m * SEQ_PER_CORE
        obs_c = obvs[s0 : s0 + SEQ_PER_CORE, :]  # [16, T]

        # x0 [128, NCOL]; col c = j*16 + b
        x0 = np.empty((128, npair, SEQ_PER_CORE), dtype=np.float64)
        x0[0:64, :, :] = uni[:, None, None]
        x0[64:128, :, :] = uni[:, None, None]
        x0[0:64, 0, :] = E[:, obs_c[:, 0]] * trans0[:, None]
        x0[64:128, -1, :] = E[:, obs_c[:, T - 1]] * w_til[:, None]
        x0 = x0.reshape(128, NCOL).astype(BF16)

        # econg [128, n_steps*NCOL] step-major, fp8
        # top: E8[:, obs[b, fwd_tok[j,i]]] -> [64, npair, n_steps, 16]
        top = E8[:, obs_c[:, fwd_tok]]  # [64, 16, npair, n_steps]
        bot = E8[:, obs_c[:, bwd_tok]]
        # reorder to [64, n_steps, npair, 16] then flatten steps-major
        top = np.transpose(top, (0, 3, 2, 1))
        bot = np.transpose(bot, (0, 3, 2, 1))
        econg = np.concatenate([top, bot], axis=0).reshape(128, n_steps * NCOL)
        per_core.append(
            {
                "wmat": wmat,
                "x0": x0,
                "econg": np.ascontiguousarray(econg),
            }
        )
    return per_core


def _host_stitch(xouts, log_trans, log_emit, obvs):
    """Assemble logZ from per-core device chain outputs (float64)."""
    log_trans = np.asarray(log_trans, dtype=np.float64)
    log_emit = np.asarray(log_emit, dtype=np.float64)
    obvs = np.asarray(obvs).astype(np.int64)
    Ttil = np.exp(log_trans[1:, 1:])
    npair = N_PAIRS

    logZ = np.zeros(B, dtype=np.float64)
    for m in range(N_CORES):
        xo = np.asarray(xouts[m], dtype=np.float64).reshape(
            128, npair, SEQ_PER_CORE
        )
        f = xo[0:64]  # [64, npair, 16]: fwd finals, block j
        zdev = xo[64:128]  # bwd finals, block l=j+1
        h = np.einsum("ij,jlb->ilb", Ttil, zdev)  # restore leading Ttil
        # stitch right-to-left: acc = h[last]; peel interior blocks
        acc = h[:, -1, :]  # [64,16] block npair (exact bwd)
        logacc = np.zeros(SEQ_PER_CORE)
        for k in range(npair - 1, 0, -1):
            # interior block k: f[:,k,:], h[:,k-1,:]
            num = np.einsum("sb,sb->b", acc, f[:, k, :])
            den = h[:, k - 1, :].mean(axis=0)  # h_k . uniform
            acc = h[:, k - 1, :]
            logacc += np.log(num) - np.log(den)
        Z = np.einsum("sb,sb->b", acc, f[:, 0, :])
        s0 = m * SEQ_PER_CORE
        logZ[s0 : s0 + SEQ_PER_CORE] = logacc + np.log(Z) - T * C_SHIFT - 99.0
    return logZ


def _run(nc, per_core, trace=False):
    from concourse.bass_utils import run_bass_kernel_spmd

    return run_bass_kernel_spmd(
        nc, per_core, list(range(N_CORES)), trace=trace, trace_cores=[0]
    )


def kernel(log_trans, log_emit, log_pi, obvs):
    n_chains = int(os.environ.get("HMM_NCHAINS", "2"))
    nc = _build_program(N_STEPS, n_chains)
    per_core = _host_prep(log_trans, log_emit, obvs, N_STEPS)
    res = _run(nc, per_core)
    xouts = [r["xout"] for r in res.results]
    out = _host_stitch(xouts, log_trans, log_emit, obvs)
    return out.astype(np.float32)


# revision 22
# speedup vs baseline: 1.1840x; 1.1840x over previous
"""Trainium2 Bass kernel: batched HMM log-forward (evidence) via segmented
rank-1 scan.

Problem: B=128 seqs, T=8192 steps, S=65 states (state 0 bookend), V=1024.
Linear-space chain:  Z*e^99 = w~^T A_8191 ... A_1 a_1,
  A_t = D_{e_t} Ttil^T,  a_1 = e_0*trans0,  w~ = exp(log_trans[1:,0]+99),
  e_t = E[:, obs[t]],    E = exp(log_emit[1:,:] + C)  (drift-compensated).

Key idea: a product of >=130 positive matrices is numerically rank-1
(Perron-Frobenius contraction; validated offline to ~1e-6), so the chain
splits into NPAIRS+1 blocks stitched by 64-dim dot products:
  * block 0 (exact fwd from a_1), block NPAIRS (exact bwd from w~), and
    NPAIRS-1 interior blocks, each contributing one fwd chain (direction
    u_k) and one bwd chain (direction v_k) from arbitrary positive inits.
  * stitch right-to-left: acc^T M_k = (acc.f_k)/(h_k.g_k) * h_k^T, where
    h_k = Ttil @ (device bwd output)  [the device bwd chain computes the
    T~-shifted product; one host matvec restores M_k^T r_k].
Serial depth drops 4096 -> 130 steps; the extra chains ride in the matmul
columns (PE/DVE have huge column headroom at the latency floor).

Device step (per chain group): one [128x128]@[128,cw] matmul -> PSUM, one
DVE multiply PSUM*e -> SBUF.  Emission stream is fp8 e5m2 (range of E is
[6e-3, 37] - all normal in e5m2; validated 4.7e-4 end-to-end), so the
entire 130-step stream fits in SBUF upfront (128KB/partition).

Sharding: data-parallel, 16 seqs per core on 8 cores.  Host prep builds
per-core streams; host epilogue stitches in float64 (microseconds).
"""

import os
import numpy as np
import ml_dtypes

B, T, S, V = 128, 8192, 65, 1024
N_CORES = 8
SEQ_PER_CORE = B // N_CORES  # 16
C_SHIFT = 6.9418
BF16 = ml_dtypes.bfloat16
FP8 = ml_dtypes.float8_e5m2

# segmentation: matrices t in [1, 8191] split into N_PAIRS+1 blocks -- the
# exact-bwd block consumes N_STEPS+1 matrices (its init absorbs one
# emission), every other block N_STEPS.
N_STEPS = int(os.environ.get("HMM_NSTEPS", "65"))
N_PAIRS = 8190 // N_STEPS - 1  # 125 for N_STEPS=65
assert (N_PAIRS + 1) * N_STEPS == 8190
NCOL = N_PAIRS * SEQ_PER_CORE  # device columns per core (1008)


def _chain_ranges(n_chains):
    """Split the NCOL columns into chain groups on pair boundaries. A
    group's PSUM tile may span multiple 2KB banks (512 fp32 cols each);
    individual matmuls are split at bank boundaries by the builder."""
    base = N_PAIRS // n_chains
    rem = N_PAIRS % n_chains
    sizes = [(base + (1 if i < rem else 0)) * SEQ_PER_CORE for i in range(n_chains)]
    ranges = []
    lo = 0
    for s in sizes:
        ranges.append((lo, lo + s))
        lo += s
    return ranges


def _strip_self_wait_events(nc):
    """Remove InstEventSemaphore instrs that only wait on the issuing
    engine's own semaphore (trivially-true WAW guards; engine execution is
    in-order). Saves sequencer slots in the scan loop."""
    eng_prefix = {
        "EngineType.DVE": "DVE_",
        "EngineType.PE": "PE_",
        "EngineType.Activation": "Activation_",
        "EngineType.Pool": "Pool_",
    }
    removed = 0
    for fn in nc.m.functions:
        for blk in fn.blocks:
            keep = []
            for inst in blk.instructions:
                if type(inst).__name__ == "InstEventSemaphore":
                    pfx = eng_prefix.get(str(getattr(inst, "engine", "")), None)
                    si = inst.sync_info
                    if (
                        pfx is not None
                        and si
                        and not si.on_update
                        and si.on_wait
                        and all(
                            w.ant_name.startswith(pfx)
                            and w.wait_mode == "sem-ge-imm"
                            for w in si.on_wait
                        )
                    ):
                        removed += 1
                        continue
                keep.append(inst)
            blk.instructions[:] = keep
    return removed


def _dedupe_ldweights(nc):
    """Drop InstLdweights reloading the identical stationary operand the PE
    already holds (weight never changes across the scan)."""
    removed = 0
    for fn in nc.m.functions:
        for blk in fn.blocks:
            last_key = None
            keep = []
            for inst in blk.instructions:
                if type(inst).__name__ == "InstLdweights":
                    si = inst.sync_info
                    clean = not si or (not si.on_wait and not si.on_update)
                    key = (
                        str(inst.ins[0]),
                        str(getattr(inst, "tile_position", None)),
                        str(getattr(inst, "perf_mode", None)),
                    )
                    if clean and key == last_key:
                        removed += 1
                        continue
                    if clean:
                        last_key = key
                    else:
                        last_key = None
                keep.append(inst)
            blk.instructions[:] = keep
    return removed


def _build_program(n_steps, n_chains):
    """SPMD Bass program: n_steps scan iterations over NCOL columns."""
    import contextlib
    import concourse.tile as tile
    from concourse import bacc, mybir

    nc = bacc.Bacc(None)
    ranges = _chain_ranges(n_chains)
    # PSUM banks hold 512 fp32 columns; a chain tile may span several banks
    # (matmul outputs must not cross a bank boundary, so split them there)
    max_cw = max(hi - lo for lo, hi in ranges)
    banks_per_tile = -(-max_cw // 512)
    psum_bufs = 2 if n_chains * banks_per_tile * 2 <= 8 else 1

    w_dram = nc.declare_dram_parameter("wmat", [128, 128], mybir.dt.bfloat16, False)
    x0_dram = nc.declare_dram_parameter("x0", [128, NCOL], mybir.dt.bfloat16, False)
    e_dram = nc.declare_dram_parameter(
        "econg", [128, n_steps * NCOL], mybir.dt.float8e5, False
    )
    out_dram = nc.declare_dram_parameter("xout", [128, NCOL], mybir.dt.bfloat16, True)

    # e-stream chunk schedule (steps per chunk): small first chunks so the
    # scan starts early, then large ones
    sched = []
    left = n_steps
    for sz in (2, 4, 8):
        if left > 0:
            sched.append(min(sz, left))
            left -= sched[-1]
    while left > 0:
        sched.append(min(16, left))
        left -= sched[-1]
    chunk_of_step = []
    for ci, sz in enumerate(sched):
        chunk_of_step += [ci] * sz
    chunk_base = np.cumsum([0] + sched[:-1])

    with tile.TileContext(nc) as tc:
        with contextlib.ExitStack() as ctx:
            const_pool = ctx.enter_context(tc.tile_pool(name="const", bufs=1))
            epool = ctx.enter_context(tc.tile_pool(name="emis", bufs=1))
            xpool = ctx.enter_context(tc.tile_pool(name="x", bufs=4))
            psum_pool = ctx.enter_context(
                tc.tile_pool(name="ps", bufs=psum_bufs, space="PSUM")
            )
            fin_pool = ctx.enter_context(tc.tile_pool(name="fin", bufs=1))

            w_sb = const_pool.tile([128, 128], mybir.dt.bfloat16, tag="w")
            nc.gpsimd.dma_start(w_sb[:], w_dram[:])
            x0_sb = const_pool.tile([128, NCOL], mybir.dt.bfloat16, tag="x0")
            nc.gpsimd.dma_start(x0_sb[:], x0_dram[:])

            e_tiles = []
            for ci, sz in enumerate(sched):
                et = epool.tile([128, sz * NCOL], mybir.dt.float8e5, tag=f"e{ci}")
                lo = int(chunk_base[ci]) * NCOL
                nc.gpsimd.dma_start(et[:], e_dram[:, lo : lo + sz * NCOL])
                e_tiles.append(et)

            dummy = fin_pool.tile([1, 4], mybir.dt.bfloat16, tag="dummy")

            xs = [(x0_sb, lo) for (lo, hi) in ranges]
            seen_chunk = -1
            for k in range(n_steps):
                ci = chunk_of_step[k]
                off = (k - int(chunk_base[ci])) * NCOL
                if ci != seen_chunk:
                    nc.vector.tensor_copy(dummy[0:1, 0:1], e_tiles[ci][0:1, 0:1])
                    seen_chunk = ci
                for ch, (lo, hi) in enumerate(ranges):
                    cw = hi - lo
                    xt, xo = xs[ch]
                    ps = psum_pool.tile([128, cw], mybir.dt.float32, tag=f"ps{ch}")
                    for sub in range(0, cw, 512):
                        se = min(cw, sub + 512)
                        nc.tensor.matmul(
                            ps[:, sub:se],
                            w_sb[:],
                            xt[:, xo + sub : xo + se],
                            start=True,
                            stop=True,
                        )
                    xn = xpool.tile([128, cw], mybir.dt.bfloat16, tag=f"x{ch}")
                    nc.vector.tensor_mul(
                        xn[:], ps[:], e_tiles[ci][:, off + lo : off + hi]
                    )
                    xs[ch] = (xn, 0)

            for ch, (lo, hi) in enumerate(ranges):
                xt, xo = xs[ch]
                nc.sync.dma_start(out_dram[:, lo:hi], xt[:, xo : xo + (hi - lo)])

    nc.compile()
    _dedupe_ldweights(nc)
    if os.environ.get("HMM_STRIP_EV", "1") == "1":
        _strip_self_wait_events(nc)
    return nc


def _host_prep(log_trans, log_emit, obvs, n_steps):
    """Per-core device inputs: weight, init columns, fp8 emission stream."""
    log_trans = np.asarray(log_trans, dtype=np.float64)
    log_emit = np.asarray(log_emit, dtype=np.float64)
    obvs = np.asarray(obvs).astype(np.int64)

    Ttil = np.exp(log_trans[1:, 1:])
    trans0 = np.exp(log_trans[0, 1:])
    w_til = np.exp(log_trans[1:, 0] + 99.0)
    E = np.exp(log_emit[1:, :] + C_SHIFT)  # [64,1024]
    E8 = E.astype(FP8)

    wmat = np.zeros((128, 128), dtype=np.float64)
    wmat[0:64, 0:64] = Ttil
    wmat[64:128, 64:128] = Ttil.T
    wmat = wmat.astype(BF16)

    npair = N_PAIRS
    # token index tables, shape [npair, n_steps], step i=1..n_steps
    i_idx = np.arange(1, n_steps + 1)
    j_idx = np.arange(npair)
    fwd_tok = n_steps * j_idx[:, None] + i_idx[None, :]  # block j: t=n*j+i
    # bwd block l=j+1: interior (l<npair): t = s_{l+1}-i = n*(l+1)+1-i
    l_idx = j_idx + 1
    bwd_tok = n_steps * (l_idx[:, None] + 1) + 1 - i_idx[None, :]
    bwd_tok[-1, :] = (T - 1) - i_idx  # exact bwd block: t = T-1-i

    per_core = []
    uni = np.full(64, 1.0 / 64, dtype=np.float64)
    for m in range(N_CORES):
        s0 = m * SEQ_PER_CORE
        obs_c = obvs[s0 : s0 + SEQ_PER_CORE, :]  # [16, T]

        # x0 [128, NCOL]; col c = j*16 + b
        x0 = np.empty((128, npair, SEQ_PER_CORE), dtype=np.float64)
        x0[0:64, :, :] = uni[:, None, None]
        x0[64:128, :, :] = uni[:, None, None]
        x0[0:64, 0, :] = E[:, obs_c[:, 0]] * trans0[:, None]
        x0[64:128, -1, :] = E[:, obs_c[:, T - 1]] * w_til[:, None]
        x0 = x0.reshape(128, NCOL).astype(BF16)

        # econg [128, n_steps*NCOL] step-major, fp8
        # top: E8[:, obs[b, fwd_tok[j,i]]] -> [64, npair, n_steps, 16]
        top = E8[:, obs_c[:, fwd_tok]]  # [64, 16, npair, n_steps]
        bot = E8[:, obs_c[:, bwd_tok]]
        # reorder to [64, n_steps, npair, 16] then flatten steps-major
        top = np.transpose(top, (0, 3, 2, 1))
        bot = np.transpose(bot, (0, 3, 2, 1))
        econg = np.concatenate([top, bot], axis=0).reshape(128, n_steps * NCOL)
        per_core.append(
            {
                "wmat": wmat,
                "x0": x0,
                "econg": np.ascontiguousarray(econg),
            }
        )
    return per_core


def _host_stitch(xouts, log_trans, log_emit, obvs):
    """Assemble logZ from per-core device chain outputs (float64)."""
    log_trans = np.asarray(log_trans, dtype=np.float64)
    log_emit = np.asarray(log_emit, dtype=np.float64)
    obvs = np.asarray(obvs).astype(np.int64)
    Ttil = np.exp(log_trans[1:, 1:])
    npair = N_PAIRS

    logZ = np.zeros(B, dtype=np.float64)
    for m in range(N_CORES):
        xo = np.asarray(xouts[m], dtype=np.float64).reshape(
            128, npair, SEQ_PER_CORE
        )
        f = xo[0:64]  # [64, npair, 16]: fwd finals, block j
        zdev = xo[64:128]  # bwd finals, block l=j+1
        h = np.einsum("ij,jlb->ilb", Ttil, zdev)  # restore leading Ttil
        # stitch right-to-left: acc = h[last]; peel interior blocks
        acc = h[:, -1, :]  # [64,16] block npair (exact bwd)
        logacc = np.zeros(SEQ_PER_CORE)
        for k in range(npair - 1, 0, -1):
            # interior block k: f[:,k,:], h[:,k-1,:]
            num = np.einsum("sb,sb->b", acc, f[:, k, :])
            den = h[:, k - 1, :].mean(axis=0)  # h_k . uniform
            acc = h[:, k - 1, :]
            logacc += np.log(num) - np.log(den)
        Z = np.einsum("sb,sb->b", acc, f[:, 0, :])
        s0 = m * SEQ_PER_CORE
        logZ[s0 : s0 + SEQ_PER_CORE] = logacc + np.log(Z) - T * C_SHIFT - 99.0
    return logZ


def _run(nc, per_core, trace=False):
    from concourse.bass_utils import run_bass_kernel_spmd

    return run_bass_kernel_spmd(
        nc, per_core, list(range(N_CORES)), trace=trace, trace_cores=[0]
    )


def kernel(log_trans, log_emit, log_pi, obvs):
    n_chains = int(os.environ.get("HMM_NCHAINS", "2"))
    nc = _build_program(N_STEPS, n_chains)
    per_core = _host_prep(log_trans, log_emit, obvs, N_STEPS)
    res = _run(nc, per_core)
    xouts = [r["xout"] for r in res.results]
    out = _host_stitch(xouts, log_trans, log_emit, obvs)
    return out.astype(np.float32)
